# revision 1
# baseline (speedup 1.0000x reference)
"""Trainium2 Bass kernel for the LocalGNOBlock (windowed GNN message passing).

Math restructuring (vs the naive 12x full MLP evaluations):
  msg first layer is linear over concat([h_i, h_j, dc]):
      z_d[i] = (A - C)[i] + (B + C)[i+d] + b1,  d in {+-1..+-6}
  where A = h @ W1a, B = h @ W1b, C = coord x w1c (rank-1).
  The second msg layer is summed over edges BEFORE the matmul:
      agg_pre = (sum_d silu(z_d)) @ W2
  Aggregate divide-by-count folds into W2 (interior count == 12) with a
  6-column fixup at each sequence end.  LayerNorm stats are computed with
  ones-vector matmuls (channel dim lives on partitions); the normalize uses
  rank-1 broadcast grids P1 = g x r, P2 = g x (mu*r) - b x 1 built on the PE.

Sharding: batch dim B=8 -> one batch element per NeuronCore (no halo needed).
Host pre/post: transpose h -> [128, N] per core (channel-major), transpose the
[128, N] output back.  Device time is what counts; host transposes are cheap.
"""

import numpy as np

K = 6
HID = 128
N = 16384
B = 8
EPS = 1e-5
T = 512                 # token chunk (matmul + elementwise granularity)
NCH = N // T            # 32 chunks
OFF0 = 8                # D_full column of token 0 (even, for bf16 alignment)
NCOL = N + 2 * OFF0     # D_full width

# offsets ordered in 4 stride-2 groups: (even uses D_A, odd uses D_B)
NEG_EVEN = [-6, -4, -2]
NEG_ODD = [-5, -3, -1]
POS_ODD = [1, 3, 5]
POS_EVEN = [2, 4, 6]
SEG_ORDER = NEG_EVEN + NEG_ODD + POS_ODD + POS_EVEN  # 12 segments in Z

_compiled = None


def _build_bass(dt_act):
    import concourse.bacc as bacc
    import concourse.bass as bass
    import concourse.tile as tile
    from concourse import mybir

    f32 = mybir.dt.float32
    DT = dt_act

    nc = bacc.Bacc("TRN2", target_bir_lowering=False, debug=False)

    # ---- DRAM I/O ----
    hT = nc.dram_tensor("hT", [HID, N], DT, kind="ExternalInput")
    coordR = nc.dram_tensor("coordR", [1, N], DT, kind="ExternalInput")
    W1a = nc.dram_tensor("W1a", [HID, HID], DT, kind="ExternalInput")
    W1b = nc.dram_tensor("W1b", [HID, HID], DT, kind="ExternalInput")
    w1c = nc.dram_tensor("w1c", [1, HID], DT, kind="ExternalInput")      # +w1c
    w1cn = nc.dram_tensor("w1cn", [1, HID], DT, kind="ExternalInput")    # -w1c
    W2s = nc.dram_tensor("W2s", [HID, HID], DT, kind="ExternalInput")     # W2/12
    U1a = nc.dram_tensor("U1a", [HID, HID], DT, kind="ExternalInput")
    U1b = nc.dram_tensor("U1b", [HID, HID], DT, kind="ExternalInput")
    U2 = nc.dram_tensor("U2", [HID, HID], DT, kind="ExternalInput")
    b1c = nc.dram_tensor("b1c", [HID, 1], f32, kind="ExternalInput")      # msg_b1
    buc = nc.dram_tensor("buc", [HID, 1], f32, kind="ExternalInput")      # upd_b1 + b2@U1b
    b2u = nc.dram_tensor("b2u", [1, HID], DT, kind="ExternalInput")      # upd_b2 row
    g_row = nc.dram_tensor("g_row", [1, HID], DT, kind="ExternalInput")  # ln_g
    nb_row = nc.dram_tensor("nb_row", [1, HID], DT, kind="ExternalInput")  # -ln_b
    ident = nc.dram_tensor("ident", [HID, HID], DT, kind="ExternalInput")
    ones_col = nc.dram_tensor("ones_col", [HID, 1], DT, kind="ExternalInput")  # 1/128
    fixf = nc.dram_tensor("fixf", [1, K], f32, kind="ExternalInput")      # 12/count head
    fixl = nc.dram_tensor("fixl", [1, K], f32, kind="ExternalInput")      # 12/count tail
    # band-select matrix: column 63 = 1/128, else 0 (stats row packing)
    selb = nc.dram_tensor("selb", [HID, 2 * 2 * NCH - 1], DT, kind="ExternalInput")
    outT = nc.dram_tensor("outT", [HID, N], f32, kind="ExternalOutput")

    Silu = mybir.ActivationFunctionType.Silu
    Sqrt = mybir.ActivationFunctionType.Sqrt

    with tile.TileContext(nc) as tc:
        with (
            tc.tile_pool(name="singles", bufs=1) as singles,
            tc.tile_pool(name="big", bufs=1) as big,
            tc.tile_pool(name="work", bufs=3) as work,
            tc.tile_pool(name="zpool", bufs=2) as zpool,
            tc.tile_pool(name="opool", bufs=3) as opool,
            tc.tile_pool(name="psA", bufs=1, space="PSUM") as psA,
            tc.tile_pool(name="psB", bufs=1, space="PSUM") as psB,
            tc.tile_pool(name="psS", bufs=1, space="PSUM") as psS,
        ):
            # ---- constants into SBUF ----
            sW1a = singles.tile([HID, HID], DT)
            sW1b = singles.tile([HID, HID], DT)
            sW2s = singles.tile([HID, HID], DT)
            sU1a = singles.tile([HID, HID], DT)
            sU1b = singles.tile([HID, HID], DT)
            sU2 = singles.tile([HID, HID], DT)
            sIdent = singles.tile([HID, HID], DT)
            for sb, dr in [(sW1a, W1a), (sW1b, W1b), (sW2s, W2s),
                           (sU1a, U1a), (sU1b, U1b), (sU2, U2), (sIdent, ident)]:
                nc.sync.dma_start(out=sb, in_=dr[:, :])
            sw1c = singles.tile([1, HID], DT)
            sw1cn = singles.tile([1, HID], DT)
            sb2u = singles.tile([1, HID], DT)
            sg = singles.tile([1, HID], DT)
            snb = singles.tile([1, HID], DT)
            for sb, dr in [(sw1c, w1c), (sw1cn, w1cn), (sb2u, b2u),
                           (sg, g_row), (snb, nb_row)]:
                nc.sync.dma_start(out=sb, in_=dr[:, :])
            sb1 = singles.tile([HID, 1], f32)
            sbu = singles.tile([HID, 1], f32)
            sones = singles.tile([HID, 1], DT)
            nc.sync.dma_start(out=sb1, in_=b1c[:, :])
            nc.sync.dma_start(out=sbu, in_=buc[:, :])
            nc.sync.dma_start(out=sones, in_=ones_col[:, :])
            # broadcast [1,6] -> [128,6] fix tiles
            sfixf = singles.tile([HID, K], f32)
            sfixl = singles.tile([HID, K], f32)
            def bcast_rows(dr):
                a = dr[0:1, :]
                return bass.AP(tensor=a.tensor, offset=a.offset,
                               ap=[[0, HID]] + list(a.ap[1:]))

            nc.gpsimd.dma_start(out=sfixf, in_=bcast_rows(fixf))
            nc.gpsimd.dma_start(out=sfixl, in_=bcast_rows(fixl))
            sones_row = singles.tile([1, T], DT)
            nc.vector.memset(sones_row, 1.0)
            ssel = singles.tile([HID, 2 * 2 * NCH - 1], DT)
            nc.sync.dma_start(out=ssel, in_=selb[:, :])

            # ---- big persistent buffers ----
            D_A = big.tile([HID, NCOL], DT)      # token j at col OFF0 + j
            D_B = big.tile([HID, NCOL], DT)      # token j at col OFF0 + 1 + j
            x_full = big.tile([HID, N], DT)
            # zero halo columns of D so boundary silu stays finite
            nc.vector.memset(D_A[:, 0:OFF0], 0.0)
            nc.vector.memset(D_A[:, OFF0 + N:NCOL], 0.0)
            nc.vector.memset(D_B[:, 0:OFF0 + 1], 0.0)
            nc.vector.memset(D_B[:, OFF0 + 1 + N:NCOL], 0.0)

            # LN stats: rows [0:32] = E[x]/chunk, [32:64] = E[x^2]/chunk
            st_ps = psS.tile([2 * NCH, T], f32)

            hts = {}
            crd = {}

            def load_chunk(c):
                ht = work.tile([HID, T], DT, tag="ht")
                nc.sync.dma_start(out=ht, in_=hT[:, c * T:(c + 1) * T])
                co = work.tile([1, T], DT, tag="co")
                nc.sync.dma_start(out=co, in_=coordR[:, c * T:(c + 1) * T])
                hts[c] = ht
                crd[c] = co

            def phase_a(c):
                # D chunk = W1b.T @ h  +  w1c x coord   (PSUM accumulate)
                d_ps = psA.tile([HID, T], f32, tag="de", bufs=2)
                nc.tensor.matmul(d_ps, sW1b, hts[c], start=True, stop=False)
                nc.tensor.matmul(d_ps, sw1c, crd[c], start=False, stop=True)
                col = OFF0 + c * T
                nc.vector.tensor_copy(D_A[:, col:col + T], d_ps)
                nc.gpsimd.tensor_copy(
                    out=D_B[:, col + 1:col + 1 + T], in_=D_A[:, col:col + T])

            def seg_in1(tile_ap, col):
                # [128, 3, T] AP over D with outer column-stride 2
                s = tile_ap[:, col:col + T]
                return bass.AP(tensor=s.tensor, offset=s.offset,
                               ap=[s.ap[0], [2, 3], [1, T]])

            def phase_b(t):
                ht, co = hts[t], crd[t]
                # E chunk = W1a.T @ h - w1c x coord
                e_ps = psA.tile([HID, T], f32, tag="de", bufs=2)
                nc.tensor.matmul(e_ps, sW1a, ht, start=True, stop=False)
                nc.tensor.matmul(e_ps, sw1cn, co, start=False, stop=True)
                e_sb = work.tile([HID, T], DT, tag="esb")
                nc.vector.tensor_copy(e_sb, e_ps)

                # Z: 12 segments of E + shifted D, 4 stride-2 groups
                z = zpool.tile([HID, 12 * T], DT, tag="z")
                zv = z.rearrange("p (s t) -> p s t", t=T)
                e_b = bass.AP(tensor=e_sb.tensor, offset=e_sb.offset,
                              ap=[e_sb.ap[0], [0, 3], [1, T]])
                base = t * T
                groups = [
                    (D_A, OFF0 + base + NEG_EVEN[0]),
                    (D_B, OFF0 + 1 + base + NEG_ODD[0]),
                    (D_B, OFF0 + 1 + base + POS_ODD[0]),
                    (D_A, OFF0 + base + POS_EVEN[0]),
                ]
                for gi, (dbuf, col) in enumerate(groups):
                    nc.vector.tensor_tensor(
                        out=zv[:, 3 * gi:3 * gi + 3, :],
                        in0=e_b, in1=seg_in1(dbuf, col),
                        op=mybir.AluOpType.add)

                # silu over all 12 segments at once (bias = msg_b1)
                nc.scalar.activation(z, z, Silu, bias=sb1, scale=1.0)

                # zero invalid boundary columns (torn edges of the sequence)
                if t == 0:
                    for s, d in enumerate(SEG_ORDER):
                        if d < 0:
                            nc.vector.memset(zv[:, s, 0:-d], 0.0)
                if t == NCH - 1:
                    for s, d in enumerate(SEG_ORDER):
                        if d > 0:
                            nc.vector.memset(zv[:, s, T - d:T], 0.0)

                # agg_pre = sum_s silu(z_s) @ W2s   (PSUM accumulation)
                a_ps = psB.tile([HID, T], f32, tag="agg")
                for s in range(12):
                    nc.tensor.matmul(a_ps, sW2s, zv[:, s, :],
                                     start=(s == 0), stop=(s == 11))
                agg = work.tile([HID, T], DT, tag="agg_sb")
                nc.vector.tensor_copy(agg, a_ps)
                if t == 0:
                    nc.vector.tensor_tensor(out=agg[:, 0:K], in0=a_ps[:, 0:K],
                                            in1=sfixf, op=mybir.AluOpType.mult)
                if t == NCH - 1:
                    nc.vector.tensor_tensor(out=agg[:, T - K:T],
                                            in0=a_ps[:, T - K:T],
                                            in1=sfixl, op=mybir.AluOpType.mult)

                # update MLP
                u_ps = psA.tile([HID, T], f32, tag="upd", bufs=2)
                nc.tensor.matmul(u_ps, sU1a, ht, start=True, stop=False)
                nc.tensor.matmul(u_ps, sU1b, agg, start=False, stop=True)
                s2 = work.tile([HID, T], DT, tag="s2")
                nc.scalar.activation(s2, u_ps, Silu, bias=sbu, scale=1.0)

                # x = h + silu@U2 + b2u  (all accumulated in PSUM)
                x_ps = psA.tile([HID, T], f32, tag="xps", bufs=2)
                nc.tensor.matmul(x_ps, sU2, s2, start=True, stop=False)
                nc.tensor.matmul(x_ps, sb2u, sones_row, start=False, stop=False)
                nc.tensor.matmul(x_ps, sIdent, ht, start=False, stop=True)
                x_sb = x_full[:, base:base + T]
                nc.vector.tensor_copy(x_sb, x_ps)
                x2 = work.tile([HID, T], DT, tag="x2")
                nc.vector.tensor_tensor(out=x2, in0=x_sb, in1=x_sb,
                                        op=mybir.AluOpType.mult)
                # LN stats rows: band-select lhsT packs E[x] into psum row t
                # and E[x^2] into row NCH+t of one accumulating [64,T] bank
                hot = 2 * NCH - 1
                nc.tensor.matmul(st_ps[:, :], ssel[:, hot - t:hot - t + 2 * NCH],
                                 x_sb, start=(t == 0), stop=False)
                nc.tensor.matmul(st_ps[:, :],
                                 ssel[:, hot - NCH - t:hot - t + NCH],
                                 x2, start=False, stop=(t == NCH - 1))

            # ---------------- pass 1 ----------------
            load_chunk(0)
            for c in range(NCH + 1):
                if c < NCH:
                    if c + 1 < NCH:
                        load_chunk(c + 1)
                    phase_a(c)
                if c >= 1:
                    phase_b(c - 1)

            # ---------------- LN stats math ----------------
            r_sb = big.tile([NCH, T], DT)       # rstd per token
            u_sb = big.tile([NCH, T], DT)       # mu * rstd per token
            ex_sb = work.tile([NCH, T], f32, tag="ex")
            nc.vector.tensor_copy(ex_sb, st_ps[0:NCH, :])
            t1 = work.tile([NCH, T], f32, tag="t1")
            nc.vector.tensor_tensor(out=t1, in0=ex_sb, in1=ex_sb,
                                    op=mybir.AluOpType.mult)
            var = work.tile([NCH, T], f32, tag="var")
            nc.vector.tensor_tensor(out=var, in0=st_ps[NCH:2 * NCH, :], in1=t1,
                                    op=mybir.AluOpType.subtract)
            seps = singles.tile([NCH, 1], f32)
            nc.vector.memset(seps, float(EPS))
            nc.scalar.activation(var, var, Sqrt, bias=seps, scale=1.0)
            with nc.allow_low_precision(reason="rstd rows feed fp16 matmuls"):
                nc.vector.reciprocal(out=r_sb, in_=var)
            nc.vector.tensor_tensor(out=u_sb, in0=ex_sb,
                                    in1=r_sb, op=mybir.AluOpType.mult)
            # ---------------- pass 2: normalize ----------------
            # K=1 matmul rhs must start at partition 0: DMA each row down
            for t in range(NCH):
                base = t * T
                rr = work.tile([1, T], DT, tag="rr")
                nc.sync.dma_start(out=rr, in_=r_sb[t:t + 1, :])
                uu = work.tile([1, T], DT, tag="uu")
                nc.sync.dma_start(out=uu, in_=u_sb[t:t + 1, :])
                p1 = psA.tile([HID, T], f32, tag="upd", bufs=2)
                nc.tensor.matmul(p1, sg, rr, start=True, stop=True)
                p2 = psA.tile([HID, T], f32, tag="xps", bufs=2)
                nc.tensor.matmul(p2, sg, uu, start=True, stop=False)
                nc.tensor.matmul(p2, snb, sones_row, start=False, stop=True)
                o = opool.tile([HID, T], f32, tag="o")
                nc.vector.tensor_tensor(out=o, in0=x_full[:, base:base + T],
                                        in1=p1, op=mybir.AluOpType.mult)
                nc.vector.tensor_tensor(out=o, in0=o, in1=p2,
                                        op=mybir.AluOpType.subtract)
                nc.sync.dma_start(out=outT[:, base:base + T], in_=o)

    nc.compile()
    return nc


def _get_compiled(dt_name):
    global _compiled
    if _compiled is None:
        from concourse import mybir
        dt = {"bf16": mybir.dt.bfloat16, "fp16": mybir.dt.float16, "fp32": mybir.dt.float32}[dt_name]
        _compiled = _build_bass(dt)
    return _compiled


DT_NAME = "fp16"


def _sel_band(act_np):
    hot = 2 * NCH - 1
    sel = np.zeros((HID, 2 * 2 * NCH - 1), dtype=np.float32)
    sel[:, hot] = 1.0 / HID
    return sel.astype(act_np)


def kernel(**inputs):
    from concourse.bass_utils import run_bass_kernel_spmd

    h = np.asarray(inputs["h"], dtype=np.float32)
    coord = np.asarray(inputs["coord"], dtype=np.float32)
    msg_w1 = np.asarray(inputs["msg_w1"], dtype=np.float32)
    msg_b1 = np.asarray(inputs["msg_b1"], dtype=np.float32)
    msg_w2 = np.asarray(inputs["msg_w2"], dtype=np.float32)
    msg_b2 = np.asarray(inputs["msg_b2"], dtype=np.float32)
    upd_w1 = np.asarray(inputs["upd_w1"], dtype=np.float32)
    upd_b1 = np.asarray(inputs["upd_b1"], dtype=np.float32)
    upd_w2 = np.asarray(inputs["upd_w2"], dtype=np.float32)
    upd_b2 = np.asarray(inputs["upd_b2"], dtype=np.float32)
    ln_g = np.asarray(inputs["ln_g"], dtype=np.float32)
    ln_b = np.asarray(inputs["ln_b"], dtype=np.float32)

    np_dt = np.dtype("bfloat16") if False else None  # placeholder
    import ml_dtypes
    act_np = {"bf16": ml_dtypes.bfloat16, "fp16": np.float16, "fp32": np.float32}[DT_NAME]

    W1a = msg_w1[:HID]
    W1b = msg_w1[HID:2 * HID]
    w1c = msg_w1[2 * HID]
    bias_u = upd_b1 + msg_b2 @ upd_w1[HID:2 * HID]
    W2s = msg_w2 / (2.0 * K)

    idx = np.arange(N)
    count = (np.minimum(idx, K) + np.minimum(N - 1 - idx, K)).astype(np.float32)
    fix = (2.0 * K) / count
    fixf = fix[:K].reshape(1, K).astype(np.float32)
    fixl = fix[N - K:].reshape(1, K).astype(np.float32)

    const = {
        "W1a": np.ascontiguousarray(W1a, dtype=act_np),
        "W1b": np.ascontiguousarray(W1b, dtype=act_np),
        "w1c": np.ascontiguousarray(w1c.reshape(1, HID), dtype=act_np),
        "w1cn": np.ascontiguousarray(-w1c.reshape(1, HID), dtype=act_np),
        "W2s": np.ascontiguousarray(W2s, dtype=act_np),
        "U1a": np.ascontiguousarray(upd_w1[:HID], dtype=act_np),
        "U1b": np.ascontiguousarray(upd_w1[HID:], dtype=act_np),
        "U2": np.ascontiguousarray(upd_w2, dtype=act_np),
        "b1c": np.ascontiguousarray(msg_b1.reshape(HID, 1), dtype=np.float32),
        "buc": np.ascontiguousarray(bias_u.reshape(HID, 1), dtype=np.float32),
        "b2u": np.ascontiguousarray(upd_b2.reshape(1, HID), dtype=act_np),
        "g_row": np.ascontiguousarray(ln_g.reshape(1, HID), dtype=act_np),
        "nb_row": np.ascontiguousarray(-ln_b.reshape(1, HID), dtype=act_np),
        "ident": np.ascontiguousarray(np.eye(HID), dtype=act_np),
        "ones_col": np.full((HID, 1), 1.0 / HID, dtype=act_np),
        "fixf": fixf,
        "fixl": fixl,
        "selb": _sel_band(act_np),
    }

    in_maps = []
    for b in range(B):
        m = dict(const)
        m["hT"] = np.ascontiguousarray(h[b].T, dtype=act_np)
        m["coordR"] = np.ascontiguousarray(coord[b].reshape(1, N), dtype=act_np)
        in_maps.append(m)

    nc = _get_compiled(DT_NAME)
    res = run_bass_kernel_spmd(nc, in_maps, core_ids=list(range(B)))
    global LAST_RESULTS
    LAST_RESULTS = res
    out = np.stack([np.asarray(res.results[b]["outT"], dtype=np.float32).T
                    for b in range(B)])
    return np.ascontiguousarray(out)



# revision 7
# speedup vs baseline: 1.7434x; 1.7434x over previous
"""Trainium2 Bass kernel for the LocalGNOBlock (windowed GNN message passing).

Math restructuring (vs the naive 12x full MLP evaluations):
  msg first layer is linear over concat([h_i, h_j, dc]):
      z_d[i] = (A - C)[i] + (B + C)[i+d] + b1,  d in {+-1..+-6}
  where A = h @ W1a, B = h @ W1b, C = coord x w1c (rank-1).
  The second msg layer is summed over edges BEFORE the matmul:
      agg_pre = (sum_d silu(z_d)) @ W2
  Aggregate divide-by-count folds into W2 (interior count == 12) with a
  6-column fixup at each sequence end.  LayerNorm stats are computed with
  band-select matmuls (channel dim lives on partitions).

Pipeline structure: 7-stage software pipeline, each consumer stage lagged
a full iteration behind its producer so no engine queue ever stalls:
  iter i: LOAD(i) | A(i)=D/E matmuls | Z(i-2)=DVE adds | S(i-3)=silu |
          G(i-4)=agg matmuls | U(i-5)=upd matmuls | X(i-6)=U2 matmul +
          x fuse | ST(i-7)=LN stats matmuls
Engine balance: Pool does the D and agg PSUM->SBUF casts, DVE does the E
cast + z adds + x fuse (scalar_tensor_tensor folds +h and +upd_b2, killing
the identity and bias rank-1 matmuls), DMA does the D_B shift copy and the
pass-2 row broadcasts (killing 3 rank-1 matmuls per chunk in pass 2).

Sharding: batch dim B=8 -> one batch element per NeuronCore (no halo).
Host pre/post: transpose h -> [128, N] per core, transpose [128, N] fp16
output back and cast to f32.
"""

import numpy as np

K = 6
HID = 128
N = 16384
B = 8
EPS = 1e-5
T = 512                 # token chunk (matmul + elementwise granularity)
NCH = N // T            # 32 chunks
OFF0 = 8                # D_full column of token 0 (even, for fp16 alignment)
NCOL = N + 2 * OFF0     # D_full width

# offsets ordered in 4 stride-2 groups: (even uses D_A, odd uses D_B)
NEG_EVEN = [-6, -4, -2]
NEG_ODD = [-5, -3, -1]
POS_ODD = [1, 3, 5]
POS_EVEN = [2, 4, 6]
SEG_ORDER = NEG_EVEN + NEG_ODD + POS_ODD + POS_EVEN  # 12 segments in Z

_compiled = None


def _build_bass(dt_act):
    import concourse.bacc as bacc
    import concourse.bass as bass
    import concourse.tile as tile
    from concourse import mybir

    f32 = mybir.dt.float32
    DT = dt_act

    nc = bacc.Bacc("TRN2", target_bir_lowering=False, debug=False)

    # ---- DRAM I/O ----
    hT = nc.dram_tensor("hT", [HID, N], DT, kind="ExternalInput")
    coordR = nc.dram_tensor("coordR", [1, N], DT, kind="ExternalInput")
    W1a = nc.dram_tensor("W1a", [HID, HID], DT, kind="ExternalInput")
    W1b = nc.dram_tensor("W1b", [HID, HID], DT, kind="ExternalInput")
    w1c = nc.dram_tensor("w1c", [1, HID], DT, kind="ExternalInput")      # +w1c
    w1cn = nc.dram_tensor("w1cn", [1, HID], DT, kind="ExternalInput")    # -w1c
    W2s = nc.dram_tensor("W2s", [HID, HID], DT, kind="ExternalInput")     # W2/12
    U1a = nc.dram_tensor("U1a", [HID, HID], DT, kind="ExternalInput")
    U1b = nc.dram_tensor("U1b", [HID, HID], DT, kind="ExternalInput")
    U2 = nc.dram_tensor("U2", [HID, HID], DT, kind="ExternalInput")
    b1c = nc.dram_tensor("b1c", [HID, 1], f32, kind="ExternalInput")      # msg_b1
    buc = nc.dram_tensor("buc", [HID, 1], f32, kind="ExternalInput")      # upd_b1 + b2@U1b
    b2c = nc.dram_tensor("b2c", [HID, 1], f32, kind="ExternalInput")      # upd_b2 col
    g_col = nc.dram_tensor("g_col", [HID, 1], f32, kind="ExternalInput")  # ln_g col
    bb_col = nc.dram_tensor("bb_col", [HID, 1], f32, kind="ExternalInput")  # ln_b col
    fixf = nc.dram_tensor("fixf", [1, K], f32, kind="ExternalInput")      # 12/count head
    fixl = nc.dram_tensor("fixl", [1, K], f32, kind="ExternalInput")      # 12/count tail
    # band-select matrix: column 2*NCH-1 = 1/128, else 0 (stats row packing)
    selb = nc.dram_tensor("selb", [HID, 2 * 2 * NCH - 1], DT, kind="ExternalInput")
    outT = nc.dram_tensor("outT", [HID, N], DT, kind="ExternalOutput")
    # DRAM bounce for the LN row stats (enables partition-broadcast reads)
    rD = nc.dram_tensor("rD", [NCH, T], DT, kind="Internal")
    uD = nc.dram_tensor("uD", [NCH, T], DT, kind="Internal")

    Silu = mybir.ActivationFunctionType.Silu
    Sqrt = mybir.ActivationFunctionType.Sqrt

    with tile.TileContext(nc) as tc:
        with (
            tc.tile_pool(name="singles", bufs=1) as singles,
            tc.tile_pool(name="big", bufs=1) as big,
            tc.tile_pool(name="work", bufs=3) as work,
            tc.tile_pool(name="zpool", bufs=3) as zpool,
            tc.tile_pool(name="opool", bufs=3) as opool,
            tc.tile_pool(name="psD", bufs=2, space="PSUM") as psD,
            tc.tile_pool(name="psE", bufs=1, space="PSUM") as psE,
            tc.tile_pool(name="psG", bufs=2, space="PSUM") as psG,
            tc.tile_pool(name="psU", bufs=1, space="PSUM") as psU,
            tc.tile_pool(name="psX", bufs=1, space="PSUM") as psX,
            tc.tile_pool(name="psS", bufs=1, space="PSUM") as psS,
        ):
            # ---- constants into SBUF ----
            sW1a = singles.tile([HID, HID], DT)
            sW1b = singles.tile([HID, HID], DT)
            sW2s = singles.tile([HID, HID], DT)
            sU1a = singles.tile([HID, HID], DT)
            sU1b = singles.tile([HID, HID], DT)
            sU2 = singles.tile([HID, HID], DT)
            for sb, dr in [(sW1a, W1a), (sW1b, W1b), (sW2s, W2s),
                           (sU1a, U1a), (sU1b, U1b), (sU2, U2)]:
                nc.sync.dma_start(out=sb, in_=dr[:, :])
            sw1c = singles.tile([1, HID], DT)
            sw1cn = singles.tile([1, HID], DT)
            for sb, dr in [(sw1c, w1c), (sw1cn, w1cn)]:
                nc.sync.dma_start(out=sb, in_=dr[:, :])
            sb1 = singles.tile([HID, 1], f32)
            sbu = singles.tile([HID, 1], f32)
            sb2 = singles.tile([HID, 1], f32)
            sg = singles.tile([HID, 1], f32)
            sbb = singles.tile([HID, 1], f32)
            nc.sync.dma_start(out=sb1, in_=b1c[:, :])
            nc.sync.dma_start(out=sbu, in_=buc[:, :])
            nc.sync.dma_start(out=sb2, in_=b2c[:, :])
            nc.sync.dma_start(out=sg, in_=g_col[:, :])
            nc.sync.dma_start(out=sbb, in_=bb_col[:, :])
            # broadcast [1,6] -> [128,6] fix tiles
            sfixf = singles.tile([HID, K], f32)
            sfixl = singles.tile([HID, K], f32)

            def bcast_rows(a):
                return bass.AP(tensor=a.tensor, offset=a.offset,
                               ap=[[0, HID]] + list(a.ap[1:]))

            nc.gpsimd.dma_start(out=sfixf, in_=bcast_rows(fixf[0:1, :]))
            nc.gpsimd.dma_start(out=sfixl, in_=bcast_rows(fixl[0:1, :]))
            ssel = singles.tile([HID, 2 * 2 * NCH - 1], DT)
            nc.sync.dma_start(out=ssel, in_=selb[:, :])

            # ---- big persistent buffers ----
            D_A = big.tile([HID, NCOL], DT)      # token j at col OFF0 + j
            D_B = big.tile([HID, NCOL], DT)      # token j at col OFF0 + 1 + j
            x_full = big.tile([HID, N], DT)
            # zero halo columns of D so boundary z stays finite
            nc.vector.memset(D_A[:, 0:OFF0], 0.0)
            nc.vector.memset(D_A[:, OFF0 + N:NCOL], 0.0)
            nc.vector.memset(D_B[:, 0:OFF0 + 1], 0.0)
            nc.vector.memset(D_B[:, OFF0 + 1 + N:NCOL], 0.0)

            # LN stats: rows [0:32] = E[x]/chunk, [32:64] = E[x^2]/chunk
            st_ps = psS.tile([2 * NCH, T], f32)

            hts = {}
            crd = {}
            zs = {}
            aggs = {}
            s2s = {}

            def stage_load(c):
                ht = work.tile([HID, T], DT, tag="ht", bufs=9)
                nc.sync.dma_start(out=ht, in_=hT[:, c * T:(c + 1) * T])
                co = work.tile([1, T], DT, tag="co", bufs=3)
                nc.sync.dma_start(out=co, in_=coordR[:, c * T:(c + 1) * T])
                hts[c] = ht
                crd[c] = co

            def stage_a(c):
                # D chunk = W1b.T @ h  +  w1c x coord   (PSUM accumulate)
                d_ps = psD.tile([HID, T], f32, tag="d")
                nc.tensor.matmul(d_ps, sW1b, hts[c], start=True, stop=False)
                nc.tensor.matmul(d_ps, sw1c, crd[c], start=False, stop=True)
                # E chunk = W1a.T @ h - w1c x coord
                e_ps = psE.tile([HID, T], f32, tag="e")
                nc.tensor.matmul(e_ps, sW1a, hts[c], start=True, stop=False)
                nc.tensor.matmul(e_ps, sw1cn, crd[c], start=False, stop=True)
                col = OFF0 + c * T
                # DVE: E and D casts (first DVE ops of the iteration)
                e_sb = work.tile([HID, T], DT, tag="esb", bufs=3)
                nc.vector.tensor_copy(e_sb, e_ps)
                nc.vector.tensor_copy(D_A[:, col:col + T], d_ps)
                hts[c] = (hts[c], e_sb)
                # DMA: D_B = D_A shifted one column right
                nc.sync.dma_start(out=D_B[:, col + 1:col + 1 + T],
                                  in_=D_A[:, col:col + T])

            def seg_in1(tile_ap, col):
                # [128, 3, T] AP over D with outer column-stride 2
                s = tile_ap[:, col:col + T]
                return bass.AP(tensor=s.tensor, offset=s.offset,
                               ap=[s.ap[0], [2, 3], [1, T]])

            def stage_z(t):
                e_sb = hts[t][1]
                # Z: 12 segments of E + shifted D, 4 stride-2 groups
                z = zpool.tile([HID, 12 * T], DT, tag="z")
                zv = z.rearrange("p (s t) -> p s t", t=T)
                e_b = bass.AP(tensor=e_sb.tensor, offset=e_sb.offset,
                              ap=[e_sb.ap[0], [0, 3], [1, T]])
                base = t * T
                groups = [
                    (D_A, OFF0 + base + NEG_EVEN[0]),
                    (D_B, OFF0 + 1 + base + NEG_ODD[0]),
                    (D_B, OFF0 + 1 + base + POS_ODD[0]),
                    (D_A, OFF0 + base + POS_EVEN[0]),
                ]
                for gi, (dbuf, col) in enumerate(groups):
                    nc.vector.tensor_tensor(
                        out=zv[:, 3 * gi:3 * gi + 3, :],
                        in0=e_b, in1=seg_in1(dbuf, col),
                        op=mybir.AluOpType.add)
                zs[t] = z

            def stage_s(t):
                z = zs[t]
                # silu over all 12 segments at once (bias = msg_b1)
                nc.scalar.activation(z, z, Silu, bias=sb1, scale=1.0)

            def stage_g(t):
                zv = zs[t].rearrange("p (s t) -> p s t", t=T)
                # agg_pre = sum_s silu(z_s) @ W2s   (PSUM accumulation)
                # boundary chunks: restrict each segment's valid column range
                # (halo D columns are zero, so silu(e) there is nonzero and
                # must be excluded); first emitted matmul must be full-width.
                segs = []
                for s, d in enumerate(SEG_ORDER):
                    lo, hi = 0, T
                    if t == 0 and d < 0:
                        lo = -d
                    if t == NCH - 1 and d > 0:
                        hi = T - d
                    segs.append((s, lo, hi))
                segs.sort(key=lambda x: (x[1] != 0) + (x[2] != T))
                a_ps = psG.tile([HID, T], f32, tag="agg")
                for k, (s, lo, hi) in enumerate(segs):
                    nc.tensor.matmul(a_ps[:, lo:hi], sW2s, zv[:, s, lo:hi],
                                     start=(k == 0), stop=(k == 11),
                                     skip_group_check=True)
                del zs[t]
                # ACT: agg cast (ACT reads PSUM cheaply)
                agg = work.tile([HID, T], DT, tag="agg_sb", bufs=3)
                nc.scalar.copy(out=agg, in_=a_ps)
                if t == 0:
                    nc.vector.tensor_tensor(out=agg[:, 0:K], in0=a_ps[:, 0:K],
                                            in1=sfixf, op=mybir.AluOpType.mult)
                if t == NCH - 1:
                    nc.vector.tensor_tensor(out=agg[:, T - K:T],
                                            in0=a_ps[:, T - K:T],
                                            in1=sfixl, op=mybir.AluOpType.mult)
                aggs[t] = agg

            def stage_u(t):
                ht = hts[t][0]
                u_ps = psU.tile([HID, T], f32, tag="u")
                nc.tensor.matmul(u_ps, sU1a, ht, start=True, stop=False)
                nc.tensor.matmul(u_ps, sU1b, aggs[t], start=False, stop=True)
                del aggs[t]
                s2 = work.tile([HID, T], DT, tag="s2", bufs=3)
                nc.scalar.activation(s2, u_ps, Silu, bias=sbu, scale=1.0)
                s2s[t] = s2

            def stage_x(t):
                ht = hts[t][0]
                base = t * T
                x_ps = psX.tile([HID, T], f32, tag="x")
                nc.tensor.matmul(x_ps, sU2, s2s[t], start=True, stop=True)
                del s2s[t]
                x_sb = x_full[:, base:base + T]
                # x = (U2@s2 + upd_b2) + h   — one DVE op, no identity matmul
                nc.vector.scalar_tensor_tensor(
                    out=x_sb, in0=x_ps, scalar=sb2, in1=ht,
                    op0=mybir.AluOpType.add, op1=mybir.AluOpType.add)
                x2 = work.tile([HID, T], DT, tag="x2", bufs=3)
                nc.gpsimd.tensor_tensor(out=x2, in0=x_sb, in1=x_sb,
                                        op=mybir.AluOpType.mult)
                del hts[t]
                return x_sb, x2

            xparts = {}

            def stage_st(t):
                x_sb, x2 = xparts.pop(t)
                # LN stats rows: band-select lhsT packs E[x] into psum row t
                # and E[x^2] into row NCH+t of one accumulating [64,T] bank
                hot = 2 * NCH - 1
                nc.tensor.matmul(st_ps[:, :], ssel[:, hot - t:hot - t + 2 * NCH],
                                 x_sb, start=(t == 0), stop=False)
                nc.tensor.matmul(st_ps[:, :],
                                 ssel[:, hot - NCH - t:hot - t + NCH],
                                 x2, start=False, stop=(t == NCH - 1))

            # ---------------- pass 1: software-pipelined loop ----------------
            for i in range(NCH + 8):
                if i < NCH:
                    stage_load(i)
                    stage_a(i)
                if 2 <= i < NCH + 2:
                    stage_z(i - 2)
                if 3 <= i < NCH + 3:
                    stage_s(i - 3)
                if 4 <= i < NCH + 4:
                    stage_g(i - 4)
                if 5 <= i < NCH + 5:
                    stage_u(i - 5)
                if 6 <= i < NCH + 6:
                    xparts[i - 6] = stage_x(i - 6)
                if 7 <= i < NCH + 7:
                    stage_st(i - 7)

            # ---------------- LN stats math ----------------
            r_sb = big.tile([NCH, T], DT)       # rstd per token
            u_sb = big.tile([NCH, T], DT)       # mu * rstd per token
            ex_sb = work.tile([NCH, T], f32, tag="ex")
            nc.vector.tensor_copy(ex_sb, st_ps[0:NCH, :])
            t1 = work.tile([NCH, T], f32, tag="t1")
            nc.vector.tensor_tensor(out=t1, in0=ex_sb, in1=ex_sb,
                                    op=mybir.AluOpType.mult)
            var = work.tile([NCH, T], f32, tag="var")
            nc.vector.tensor_tensor(out=var, in0=st_ps[NCH:2 * NCH, :], in1=t1,
                                    op=mybir.AluOpType.subtract)
            seps = singles.tile([NCH, 1], f32)
            nc.vector.memset(seps, float(EPS))
            nc.scalar.activation(var, var, Sqrt, bias=seps, scale=1.0)
            with nc.allow_low_precision(reason="rstd rows feed fp16 math"):
                nc.vector.reciprocal(out=r_sb, in_=var)
            nc.vector.tensor_tensor(out=u_sb, in0=ex_sb,
                                    in1=r_sb, op=mybir.AluOpType.mult)
            nc.sync.dma_start(out=rD[:, :], in_=r_sb)
            nc.sync.dma_start(out=uD[:, :], in_=u_sb)

            # ---------------- pass 2: normalize ----------------
            # out = ((x*R - U) * g) + b, with R/U built by DMA row-broadcast
            for t in range(NCH):
                base = t * T
                Rb = opool.tile([HID, T], DT, tag="rb", bufs=3)
                nc.gpsimd.dma_start(out=Rb, in_=bcast_rows(rD[t:t + 1, :]))
                Ub = opool.tile([HID, T], DT, tag="ub", bufs=3)
                nc.gpsimd.dma_start(out=Ub, in_=bcast_rows(uD[t:t + 1, :]))
                o1 = opool.tile([HID, T], DT, tag="o1", bufs=3)
                nc.vector.tensor_tensor(out=o1, in0=x_full[:, base:base + T],
                                        in1=Rb, op=mybir.AluOpType.mult)
                nc.vector.tensor_tensor(out=o1, in0=o1, in1=Ub,
                                        op=mybir.AluOpType.subtract)
                oo = opool.tile([HID, T], DT, tag="oo", bufs=3)
                nc.vector.tensor_scalar(out=oo, in0=o1, scalar1=sg, scalar2=sbb,
                                        op0=mybir.AluOpType.mult,
                                        op1=mybir.AluOpType.add)
                nc.sync.dma_start(out=outT[:, base:base + T], in_=oo)

    nc.compile()
    return nc


def _get_compiled(dt_name):
    global _compiled
    if _compiled is None:
        from concourse import mybir
        dt = {"bf16": mybir.dt.bfloat16, "fp16": mybir.dt.float16, "fp32": mybir.dt.float32}[dt_name]
        _compiled = _build_bass(dt)
    return _compiled


DT_NAME = "fp16"


def _sel_band(act_np):
    hot = 2 * NCH - 1
    sel = np.zeros((HID, 2 * 2 * NCH - 1), dtype=np.float32)
    sel[:, hot] = 1.0 / HID
    return sel.astype(act_np)


def kernel(**inputs):
    from concourse.bass_utils import run_bass_kernel_spmd

    h = np.asarray(inputs["h"], dtype=np.float32)
    coord = np.asarray(inputs["coord"], dtype=np.float32)
    msg_w1 = np.asarray(inputs["msg_w1"], dtype=np.float32)
    msg_b1 = np.asarray(inputs["msg_b1"], dtype=np.float32)
    msg_w2 = np.asarray(inputs["msg_w2"], dtype=np.float32)
    msg_b2 = np.asarray(inputs["msg_b2"], dtype=np.float32)
    upd_w1 = np.asarray(inputs["upd_w1"], dtype=np.float32)
    upd_b1 = np.asarray(inputs["upd_b1"], dtype=np.float32)
    upd_w2 = np.asarray(inputs["upd_w2"], dtype=np.float32)
    upd_b2 = np.asarray(inputs["upd_b2"], dtype=np.float32)
    ln_g = np.asarray(inputs["ln_g"], dtype=np.float32)
    ln_b = np.asarray(inputs["ln_b"], dtype=np.float32)

    import ml_dtypes
    act_np = {"bf16": ml_dtypes.bfloat16, "fp16": np.float16, "fp32": np.float32}[DT_NAME]

    W1a = msg_w1[:HID]
    W1b = msg_w1[HID:2 * HID]
    w1c = msg_w1[2 * HID]
    bias_u = upd_b1 + msg_b2 @ upd_w1[HID:2 * HID]
    W2s = msg_w2 / (2.0 * K)

    idx = np.arange(N)
    count = (np.minimum(idx, K) + np.minimum(N - 1 - idx, K)).astype(np.float32)
    fix = (2.0 * K) / count
    fixf = fix[:K].reshape(1, K).astype(np.float32)
    fixl = fix[N - K:].reshape(1, K).astype(np.float32)

    const = {
        "W1a": np.ascontiguousarray(W1a, dtype=act_np),
        "W1b": np.ascontiguousarray(W1b, dtype=act_np),
        "w1c": np.ascontiguousarray(w1c.reshape(1, HID), dtype=act_np),
        "w1cn": np.ascontiguousarray(-w1c.reshape(1, HID), dtype=act_np),
        "W2s": np.ascontiguousarray(W2s, dtype=act_np),
        "U1a": np.ascontiguousarray(upd_w1[:HID], dtype=act_np),
        "U1b": np.ascontiguousarray(upd_w1[HID:], dtype=act_np),
        "U2": np.ascontiguousarray(upd_w2, dtype=act_np),
        "b1c": np.ascontiguousarray(msg_b1.reshape(HID, 1), dtype=np.float32),
        "buc": np.ascontiguousarray(bias_u.reshape(HID, 1), dtype=np.float32),
        "b2c": np.ascontiguousarray(upd_b2.reshape(HID, 1), dtype=np.float32),
        "g_col": np.ascontiguousarray(ln_g.reshape(HID, 1), dtype=np.float32),
        "bb_col": np.ascontiguousarray(ln_b.reshape(HID, 1), dtype=np.float32),
        "fixf": fixf,
        "fixl": fixl,
        "selb": _sel_band(act_np),
    }

    in_maps = []
    for b in range(B):
        m = dict(const)
        m["hT"] = np.ascontiguousarray(h[b].T, dtype=act_np)
        m["coordR"] = np.ascontiguousarray(coord[b].reshape(1, N), dtype=act_np)
        in_maps.append(m)

    nc = _get_compiled(DT_NAME)
    res = run_bass_kernel_spmd(nc, in_maps, core_ids=list(range(B)))
    global LAST_RESULTS
    LAST_RESULTS = res
    out = np.stack([np.asarray(res.results[b]["outT"], dtype=np.float32).T
                    for b in range(B)])
    return np.ascontiguousarray(out)
